# revision 1
# baseline (speedup 1.0000x reference)
"""Trainium2 Bass kernel for nn_CausalLiquidNetworkModel (B=256, S=2048, I=1, H=128, O=1).

Strategy: pure data parallel over batch — 8 NeuronCores, 32 batch columns
each; all parameters (<70 KB) replicated. The sequential 2048-step liquid
recurrence runs locally per core in a fully unrolled Tile program.

Math (s = DT/clip(tau, .1, 10), per step t):
  ew_t  = sigmoid(w0*x_t + w1*x_{t-1} + b_ev),  ew_0 = 0        [precomputed]
  ic_t  = tanh(w_in*x_t + b_in)                                  [bulk, chunked]
  att_t = sigmoid(W_att @ h_t + b_att)
  rec_t = tanh(W_rec @ (h_t * att_t) + b_rec)
  h_{t+1} = t1_t + t3_t
      t1_t = (rec_t * s) * E_t          E = (1+ew) partition-broadcast
      t3_t = G_t*h_t + P_t              G = 1 - s*E,  P = s*E*ic  [bulk]
  y_t = tanh(W_acc @ h_{t+1} + b_acc);  acc_{t+1} = 0.9*acc_t + 0.1*ew_t*y_t
  out = W_out @ (h_f + acc_f) + b_out

Device layout: state [H=128 partitions, B_local=32 free]; weights stationary
(lhsT = W.T).  z_att is accumulated in PSUM from the two h' addends (t3 ready
early, t1 on the critical path) so the h'-sum itself stays off the chain.  The
acc branch for step t-1 is emitted in period t, slotted into engine idle gaps;
per-chunk E/EW broadcasts come in via strided stride-0 DMA from DRAM scratch.
Engine placement keeps the serial chain on DVE/PE/ACT only and pushes t3 to
GPSIMD; same-engine scheduling pins keep ready-early side work out of the
chain's dispatch slots.
"""
import numpy as np

import concourse.bacc as bacc
import concourse.bass as bass
import concourse.tile as tile
from concourse import mybir
from concourse.tile_rust import add_dep_helper

F32 = mybir.dt.float32
MULT = mybir.AluOpType.mult
ADD = mybir.AluOpType.add
AF = mybir.ActivationFunctionType
DT = 0.1
H = 128
NCORES = 8


def _build(S=2048, BL=32, CH=64, NSLICE=16):
    assert S % CH == 0 and CH >= NSLICE
    nchunk = S // CH
    bsl = max(1, BL // NSLICE)
    nc = bacc.Bacc("TRN2", target_bir_lowering=False, debug=False,
                   num_devices=NCORES)

    x_d = nc.dram_tensor("x", [BL, S], F32, kind="ExternalInput").ap()
    wTa_d = nc.dram_tensor("wTa", [H, H], F32, kind="ExternalInput").ap()
    wTr_d = nc.dram_tensor("wTr", [H, H], F32, kind="ExternalInput").ap()
    wTc_d = nc.dram_tensor("wTc", [H, H], F32, kind="ExternalInput").ap()
    # cvec cols: 0 b_att, 1 b_rec, 2 b_acc, 3 b_in, 4 w_in, 5 s, 6 -s, 7 w_outT
    cv_d = nc.dram_tensor("cvec", [H, 8], F32, kind="ExternalInput").ap()
    # cvb cols (replicated over BL rows): 0 w1, 1 w0, 2 b_ev, 3 b_out
    cvb_d = nc.dram_tensor("cvb", [BL, 4], F32, kind="ExternalInput").ap()

    out_d = nc.dram_tensor("out", [1, BL], F32, kind="ExternalOutput").ap()
    ew_d = nc.dram_tensor("ew", [BL, S], F32, kind="ExternalOutput").ap()

    with tile.TileContext(nc) as tc:
        with (
            tc.tile_pool(name="consts", bufs=1) as consts,
            tc.tile_pool(name="pre", bufs=1) as pre,
            tc.tile_pool(name="dram", bufs=1, space="DRAM") as dpool,
            tc.tile_pool(name="chunk", bufs=2) as chunk,
            tc.tile_pool(name="step", bufs=6) as step,
            tc.tile_pool(name="state", bufs=3) as state,
            tc.tile_pool(name="pa", bufs=2, space="PSUM") as pa,
            tc.tile_pool(name="pr", bufs=3, space="PSUM") as pr,
            tc.tile_pool(name="pc", bufs=2, space="PSUM") as pcp,
            tc.tile_pool(name="po", bufs=1, space="PSUM") as po,
        ):
            # ---- constants ----
            wTa = consts.tile([H, H], F32, tag="wTa")
            wTr = consts.tile([H, H], F32, tag="wTr")
            wTc = consts.tile([H, H], F32, tag="wTc")
            cv = consts.tile([H, 8], F32, tag="cv")
            cvb = consts.tile([BL, 4], F32, tag="cvb")
            nc.sync.dma_start(out=wTa, in_=wTa_d)
            nc.sync.dma_start(out=wTr, in_=wTr_d)
            nc.sync.dma_start(out=wTc, in_=wTc_d)
            nc.sync.dma_start(out=cv, in_=cv_d)
            nc.sync.dma_start(out=cvb, in_=cvb_d)
            b_att, b_rec, b_acc, b_in = (cv[:, i : i + 1] for i in range(4))
            w_in, s_ap, ns_ap, w_out = (cv[:, i : i + 1] for i in range(4, 8))
            w1_ap, w0_ap, bev_ap = (cvb[:, i : i + 1] for i in range(3))
            bout_ap = cvb[0:1, 3:4]

            # ---- one-time: ew over [BL, S] ----
            x_sb = pre.tile([BL, S], F32, tag="x_sb")
            nc.sync.dma_start(out=x_sb, in_=x_d)
            t1p = pre.tile([BL, S - 1], F32, tag="t1p")
            nc.vector.tensor_scalar(t1p, x_sb[:, : S - 1], w1_ap, None, MULT)
            t2p = pre.tile([BL, S - 1], F32, tag="t2p")
            nc.vector.scalar_tensor_tensor(t2p, x_sb[:, 1:], w0_ap, t1p, MULT, ADD)
            ew_sb = pre.tile([BL, S], F32, tag="ew_sb")
            nc.vector.memset(ew_sb[:, 0:1], 0.0)
            nc.scalar.activation(ew_sb[:, 1:], t2p, AF.Sigmoid, bias=bev_ap)
            e1_sb = pre.tile([BL, S], F32, tag="e1_sb")
            nc.vector.tensor_scalar(e1_sb, ew_sb, 1.0, None, ADD)
            nc.sync.dma_start(out=ew_d, in_=ew_sb)
            ews_sc = dpool.tile([BL, S], F32, tag="ews_sc")
            e1s_sc = dpool.tile([BL, S], F32, tag="e1s_sc")
            nc.sync.dma_start(out=ews_sc, in_=ew_sb)
            nc.sync.dma_start(out=e1s_sc, in_=e1_sb)

            def bcast_src(dram_ap, col0):
                # [1, chunk] row block -> [H, BL, CH] partition broadcast
                return bass.AP(
                    tensor=dram_ap.tensor,
                    offset=dram_ap.offset + col0,
                    ap=[[0, H], [S, BL], [1, CH]],
                )

            def chunk_dmas(c):
                Ec = chunk.tile([H, BL, CH], F32, tag="Ec")
                EWc = chunk.tile([H, BL, CH], F32, tag="EWc")
                xbc = chunk.tile([H, BL, CH], F32, tag="xbc")
                nc.sync.dma_start(out=Ec, in_=bcast_src(e1s_sc, c * CH))
                nc.sync.dma_start(out=EWc, in_=bcast_src(ews_sc, c * CH))
                nc.sync.dma_start(out=xbc, in_=bcast_src(x_d, c * CH))
                icc = chunk.tile([H, BL, CH], F32, tag="icc")
                Gc = chunk.tile([H, BL, CH], F32, tag="Gc")
                Pc = chunk.tile([H, BL, CH], F32, tag="Pc")
                return dict(E=Ec, EW=EWc, xb=xbc, ic=icc, G=Gc, P=Pc)

            def chunk_slice(d, j, pin_act=None, pin_dve=None):
                sl = (slice(None), slice(j * bsl, (j + 1) * bsl), slice(None))
                ic_i = nc.scalar.activation(
                    d["ic"][sl], d["xb"][sl], AF.Tanh, bias=b_in, scale=w_in
                )
                g_i = nc.vector.tensor_scalar(
                    d["G"][sl], d["E"][sl], ns_ap, 1.0, MULT, ADD
                )
                p_i = nc.vector.scalar_tensor_tensor(
                    d["P"][sl], d["ic"][sl], s_ap, d["E"][sl], MULT, MULT
                )
                if pin_act is not None:
                    add_dep_helper(ic_i.ins, pin_act.ins, False, "bulk-act-after")
                if pin_dve is not None:
                    add_dep_helper(g_i.ins, pin_dve.ins, False, "bulk-g-after")
                add_dep_helper(p_i.ins, g_i.ins, False, "bulk-p-after-g")

            nslice_real = BL // bsl
            cur = chunk_dmas(0)
            for j in range(nslice_real):
                chunk_slice(cur, j)
            nxt = None

            # ---- state init ----
            h_cur = state.tile([H, BL], F32, tag="h")
            acc_cur = state.tile([H, BL], F32, tag="acc")
            t1_prev = state.tile([H, BL], F32, tag="t1")
            t3_prev = state.tile([H, BL], F32, tag="t3")
            nc.vector.memset(h_cur, 0.0)
            nc.vector.memset(acc_cur, 0.0)
            nc.vector.memset(t1_prev, 0.0)
            nc.vector.memset(t3_prev, 0.0)

            prev_chunk = None

            # ---- the scan ----
            for t in range(S):
                c, tt = divmod(t, CH)
                if tt == 0 and c > 0:
                    prev_chunk, cur = cur, nxt
                    nxt = None

                # z_att accumulated from prev step's h' addends
                z_att = pa.tile([H, BL], F32, tag="z_att")
                nc.tensor.matmul(z_att, wTa, t3_prev, start=True, stop=False)
                nc.tensor.matmul(z_att, wTa, t1_prev, start=False, stop=True)
                att = step.tile([H, BL], F32, tag="att")
                sig_i = nc.scalar.activation(att, z_att, AF.Sigmoid, bias=b_att)

                # acc-branch matmul+tanh for step t-1 (h_cur == h_t)
                if t >= 1:
                    z_acc = pcp.tile([H, BL], F32, tag="z_acc")
                    nc.tensor.matmul(z_acc, wTc, h_cur, start=True, stop=True)
                    y = step.tile([H, BL], F32, tag="y")
                    y_i = nc.scalar.activation(y, z_acc, AF.Tanh, bias=b_acc)
                    add_dep_helper(y_i.ins, sig_i.ins, False, "y-after-sigma")
                    pt, ptt = divmod(t - 1, CH)
                    EW_t1 = (prev_chunk if tt == 0 else cur)["EW"][:, :, ptt]

                # off-path: t3_t on gpsimd
                t2 = step.tile([H, BL], F32, tag="t2")
                nc.gpsimd.tensor_mul(t2, cur["G"][:, :, tt], h_cur)
                t3 = step.tile([H, BL], F32, tag="t3s")
                nc.gpsimd.tensor_add(t3, t2, cur["P"][:, :, tt])

                # on path
                ha = step.tile([H, BL], F32, tag="ha")
                ha_i = nc.vector.tensor_mul(ha, h_cur, att)
                z_rec = pr.tile([H, BL], F32, tag="z_rec")
                nc.tensor.matmul(z_rec, wTr, ha, start=True, stop=True)
                rec = step.tile([H, BL], F32, tag="rec")
                rec_i = nc.scalar.activation(rec, z_rec, AF.Tanh, bias=b_rec)
                t1 = step.tile([H, BL], F32, tag="t1s")
                nc.vector.scalar_tensor_tensor(
                    t1, rec, s_ap, cur["E"][:, :, tt], MULT, MULT
                )
                h_next = state.tile([H, BL], F32, tag="h")
                hn_i = nc.vector.tensor_add(h_next, t1, t3)

                # rest of acc-branch (DVE), pinned behind the h' update
                if t >= 1:
                    t4 = step.tile([H, BL], F32, tag="t4")
                    t4_i = nc.vector.scalar_tensor_tensor(t4, y, 0.1, EW_t1, MULT, MULT)
                    add_dep_helper(t4_i.ins, hn_i.ins, False, "t4-after-hnext")
                    acc_new = state.tile([H, BL], F32, tag="acc")
                    nc.vector.scalar_tensor_tensor(acc_new, acc_cur, 0.9, t4, MULT, ADD)
                    acc_cur = acc_new

                # next chunk: DMAs at the bottom of the tt==0 iteration, bulk
                # slices spread over following steps in engine idle windows
                if tt == 0 and c + 1 < nchunk:
                    nxt = chunk_dmas(c + 1)
                if nxt is not None and 1 <= tt <= nslice_real:
                    chunk_slice(nxt, tt - 1, pin_act=rec_i, pin_dve=ha_i)

                t1_prev, t3_prev, h_cur = t1, t3, h_next

            # ---- epilogue: acc for t=S-1, then output ----
            z_acc = pcp.tile([H, BL], F32, tag="z_acc")
            nc.tensor.matmul(z_acc, wTc, h_cur, start=True, stop=True)
            y = step.tile([H, BL], F32, tag="y")
            nc.scalar.activation(y, z_acc, AF.Tanh, bias=b_acc)
            t4 = step.tile([H, BL], F32, tag="t4")
            nc.vector.scalar_tensor_tensor(t4, y, 0.1, cur["EW"][:, :, CH - 1], MULT, MULT)
            acc_f = state.tile([H, BL], F32, tag="acc")
            nc.vector.scalar_tensor_tensor(acc_f, acc_cur, 0.9, t4, MULT, ADD)

            hs = step.tile([H, BL], F32, tag="hs")
            nc.vector.tensor_add(hs, h_cur, acc_f)
            z_out = po.tile([1, BL], F32, tag="z_out")
            nc.tensor.matmul(z_out, w_out, hs, start=True, stop=True)
            outs = step.tile([1, BL], F32, tag="outs")
            nc.scalar.activation(outs, z_out, AF.Identity, bias=bout_ap)
            nc.sync.dma_start(out=out_d, in_=outs)

    nc.compile()
    return nc


_CACHE = {}


def _get_nc(S, BL, CH=64):
    key = (S, BL, CH)
    if key not in _CACHE:
        _CACHE[key] = _build(S, BL, CH)
    return _CACHE[key]


def _make_in_maps(g, S, BL):
    tau_c = np.clip(g["tau"], 0.1, 10.0)
    s = (DT / tau_c).astype(np.float32)
    cvec = np.stack(
        [
            g["b_att"], g["b_rec"], g["b_acc"], g["b_in"],
            g["W_in"][:, 0], s, -s, g["W_out"][0, :],
        ],
        axis=1,
    ).astype(np.float32)
    w0 = float(g["W_ev"][0, 0])
    w1 = float(g["W_ev"][0, 1])
    bev = float(g["b_ev"][0])
    bout = float(g["b_out"][0])
    cvb = np.tile(np.array([[w1, w0, bev, bout]], dtype=np.float32), (BL, 1))

    xs = np.ascontiguousarray(g["x"][:, :S, 0])  # [B, S]
    wTa = np.ascontiguousarray(g["W_att"].T)
    wTr = np.ascontiguousarray(g["W_rec"].T)
    wTc = np.ascontiguousarray(g["W_acc"].T)

    return [
        {
            "x": np.ascontiguousarray(xs[i * BL : (i + 1) * BL]),
            "wTa": wTa, "wTr": wTr, "wTc": wTc,
            "cvec": cvec, "cvb": cvb,
        }
        for i in range(NCORES)
    ]


def kernel(**inputs):
    """Full inputs (as from setup_inputs) -> (output [B,1], event_weights [B,S])."""
    from concourse.bass_utils import run_bass_kernel_spmd

    g = {k: np.asarray(v, dtype=np.float32) for k, v in inputs.items()}
    B, S, _ = g["x"].shape
    BL = B // NCORES
    nc = _get_nc(S, BL)
    in_maps = _make_in_maps(g, S, BL)
    res = run_bass_kernel_spmd(nc, in_maps, core_ids=list(range(NCORES)))
    out = np.concatenate([r["out"].T for r in res.results], axis=0)  # [B, 1]
    ew = np.concatenate([r["ew"] for r in res.results], axis=0)      # [B, S]
    return out.astype(np.float32), ew.astype(np.float32)
